# revision 17
# baseline (speedup 1.0000x reference)
"""Locally-connected convolution (unshared weights) on 8 Trainium2 NeuronCores.

out[b,o,i,j] = sum_{c,u,v} x[b,c,i+u,j+v] * weight[i,j,o,c,u,v]
  x: [64, 64, 32, 32] f32, weight: [28, 28, 128, 64, 5, 5] f32 -> out [64, 128, 28, 28]

Strategy: each of the 784 output positions is an independent GEMM
[B=64, K=1600] x [K=1600, O=128].  Shard the 784 positions across 8 cores
(98 each, raster-contiguous).  Weights are reordered host-side into
K-chunk-major fp16 layout; x is pre-transposed/sheared host-side so that a
single SPMD program (identical AP offsets on every core) can slice 2-tap
K=128 lhsT tiles straight out of SBUF with zero on-device data movement.

K ordering (1600 = 25 taps x 64 ch, padded to 13 chunks x 128):
  chunk kc in [0,10): taps (u, ve) and (u, ve+1), u=kc//2, ve=2*(kc%2)
  chunk 10: taps (0,4),(1,4);  chunk 11: taps (2,4),(3,4)
  chunk 12: tap (4,4) alone (K=64)
The second tap of a chunk differs from the first by +1 column (chunks 0-9)
or +1 row (chunks 10-11) in the input plane, so partitions 64-127 of the
SBUF x-workspace hold a copy of x shifted by +1 column (region W1) or +1
row (region W2); a [128, 64] lhsT slice then covers both taps at once.
"""

import numpy as np

B, C, H, W = 64, 64, 32, 32
ROWS = COLS = 28
O, KH, KW = 128, 5, 5
NCORES = 8
PPC = (ROWS * COLS) // NCORES  # 98 positions per core
NKC = 13                       # K chunks of 128 (last is half)
KIN = 128
XROWS, XW = 8, 36              # sheared x grid: 8 input rows x 36 cols
XFLAT = XROWS * XW * B         # 18432 elements per channel-partition
SH_W1 = B                      # +1 column shift, in elements
SH_W2 = XW * B                 # +1 row shift
NPOS_BLK = 7
NBLK = PPC // NPOS_BLK         # 14 weight blocks per core
BLOCKS = [2, 3, 4, 5] + [4] * 21   # ramp-friendly block sizes (sum = 98)
FPOS = NKC * O                 # weight elements per partition per position

# kc -> (du, dv, region)  AP slot offset = ((di+du)*XW + (w2+dv))*B + region*XFLAT
CHUNK_OFF = [(kc // 2, 2 * (kc % 2), 0) for kc in range(10)] + [
    (0, 4, 1),
    (2, 4, 1),
    (4, 4, 0),  # lone tap (4,4): K=64, lower partitions only
]
# kc -> ((u0,v0), (u1,v1) or None)
CHUNK_TAPS = [((kc // 2, 2 * (kc % 2)), (kc // 2, 2 * (kc % 2) + 1)) for kc in range(10)] + [
    ((0, 4), (1, 4)),
    ((2, 4), (3, 4)),
    ((4, 4), None),
]


def _core_geom(k):
    p0 = PPC * k
    return p0 // COLS, p0 % COLS  # r0 (first input/output row), s0 in {0, 14}


def _pos_slot(t):
    """Relative position t in [0,98) -> (di, w2) grid coords shared by all cores."""
    di, jj = t // COLS, t % COLS
    return di, jj + (4 if jj >= 14 else 0)


def _build_xs(x_chwb, k):
    """x_chwb: [C,H,W,B] -> sheared per-core workspace [C, XROWS, XW, B]."""
    r0, s0 = _core_geom(k)
    xs = np.zeros((C, XROWS, XW, B), dtype=x_chwb.dtype)
    for h in range(XROWS):
        if s0 == 0:
            xs[:, h, 0:18] = x_chwb[:, r0 + h, 0:18]
            xs[:, h, 18:36] = x_chwb[:, r0 + h, 14:32]
        else:
            xs[:, h, 0:18] = x_chwb[:, r0 + h, 14:32]
            if r0 + h + 1 < H:
                xs[:, h, 18:36] = x_chwb[:, r0 + h + 1, 0:18]
    return xs.reshape(C, XFLAT)


def _abs_pos(k, t):
    p = PPC * k + t
    return p // COLS, p % COLS


def _build_wt(weight, k, dtype=np.float16):
    """weight [ROWS,COLS,O,C,KH,KW] -> per-core [KIN, PPC, NKC*O] in chunk layout."""
    ii, jj = zip(*[_abs_pos(k, t) for t in range(PPC)])
    wc = weight[list(ii), list(jj)]  # [PPC, O, C, KH, KW]
    uu = np.zeros((KIN, NKC), np.int64)
    vv = np.zeros((KIN, NKC), np.int64)
    cc = np.broadcast_to((np.arange(KIN) % C)[:, None], (KIN, NKC))
    valid = np.ones((KIN, NKC), bool)
    for kc in range(NKC):
        t0, t1 = CHUNK_TAPS[kc]
        uu[:C, kc], vv[:C, kc] = t0
        if t1 is None:
            valid[C:, kc] = False
        else:
            uu[C:, kc], vv[C:, kc] = t1
    # [PPC, O, KIN, NKC]
    wt = wc[:, :, cc, uu, vv]
    wt[:, :, ~valid] = 0
    # -> [KIN, PPC*NKC*O] position-major contiguous
    return np.ascontiguousarray(wt.transpose(2, 0, 3, 1)).reshape(KIN, PPC * NKC * O).astype(dtype)


def _emulate_core(xs_flat, wt, out_dtype=np.float32):
    """Pure-numpy emulation of the device program (mirrors AP arithmetic)."""
    wt = wt.reshape(KIN, PPC, NKC * O)
    out = np.zeros((PPC, B, O), out_dtype)
    for t in range(PPC):
        di, w2 = _pos_slot(t)
        acc = np.zeros((B, O), np.float32)
        for kc in range(NKC):
            du, dv, reg = CHUNK_OFF[kc]
            s = ((di + du) * XW + (w2 + dv)) * B
            lo = xs_flat[:, s:s + B]
            rhs = wt[:, t, kc * O:(kc + 1) * O]
            if CHUNK_TAPS[kc][1] is not None:
                sh = SH_W1 if reg == 0 else SH_W2
                assert s + sh + B <= XFLAT, (t, kc, s)
                hi = xs_flat[:, s + sh:s + sh + B]
                lhsT = np.concatenate([lo, hi], axis=0)
            else:
                lhsT = lo
                rhs = rhs[:C]
            acc += lhsT.astype(np.float32).T @ rhs.astype(np.float32)
        out[t] = acc
    return out


def _assemble(outs):
    """list of 8 per-core [PPC, B, O] -> [B, O, ROWS, COLS] f32."""
    full = np.concatenate([np.asarray(o, np.float32) for o in outs], axis=0)
    return np.ascontiguousarray(full.transpose(1, 2, 0)).reshape(B, O, ROWS, COLS)


_PROG_CACHE = {}


def _build_program():
    if "nc" in _PROG_CACHE:
        return _PROG_CACHE["nc"]
    import concourse.bass as bass
    import concourse.tile as tile
    from concourse import bacc, mybir

    f16, f32 = mybir.dt.float16, mybir.dt.float32
    XWB = XW * B      # one sheared input row, elements per partition
    HALF = 18 * B     # half a sheared row (positions j<14 touch only half 0)
    nc = bacc.Bacc("TRN2", target_bir_lowering=False, debug=False, num_devices=NCORES)
    xs1_d = nc.dram_tensor("xs1", [XROWS, 128, XWB], f16, kind="ExternalInput")
    xs2_d = nc.dram_tensor("xs2", [XROWS, 128, XWB], f16, kind="ExternalInput")
    wt_d = nc.dram_tensor("wt", [KIN, PPC * FPOS], f16, kind="ExternalInput")
    out_d = nc.dram_tensor("out", [PPC, B, O], f16, kind="ExternalOutput")

    with tile.TileContext(nc) as tc:
        with tc.tile_pool(name="xpool", bufs=1) as xpool, \
             tc.tile_pool(name="wpool", bufs=5) as wpool, \
             tc.tile_pool(name="opool", bufs=4) as opool, \
             tc.tile_pool(name="psum", bufs=8, space="PSUM") as ppool:
            xs1, xs2 = xs1_d.ap(), xs2_d.ap()
            XH = [[xpool.tile([128, 2 * HALF], f16, name=f"xh{h}_{hf}", tag=f"xh{h}_{hf}")
                   for hf in range(2)] for h in range(XROWS)]

            def load_xrow(h, hf):
                nc.sync.dma_start(XH[h][hf][:, 0:HALF], xs1[h, :, hf * HALF:(hf + 1) * HALF])
                nc.scalar.dma_start(XH[h][hf][:, HALF:2 * HALF], xs2[h, :, hf * HALF:(hf + 1) * HALF])

            wt_ap = wt_d.ap()
            out_ap = out_d.ap()
            eng = [nc.sync, nc.scalar]
            t0s = [sum(BLOCKS[:i]) for i in range(len(BLOCKS))]
            wtiles = [wpool.tile([KIN, n * FPOS], f16, name=f"wt{i}", tag="wt")
                      for i, n in enumerate(BLOCKS)]

            def load_wblk(i):
                t0, n = t0s[i], BLOCKS[i]
                n0 = (n + 1) // 2
                eng[i % 2].dma_start(
                    wtiles[i][:, 0:n0 * FPOS],
                    wt_ap[:, t0 * FPOS:(t0 + n0) * FPOS])
                eng[(i + 1) % 2].dma_start(
                    wtiles[i][:, n0 * FPOS:],
                    wt_ap[:, (t0 + n0) * FPOS:(t0 + n) * FPOS])

            # emission order ~= per-ring FIFO order: first MMs (block 0: j=0..2,
            # row 0) need x rows 0-4 half 0 plus a 3-position weight tile.
            for h in range(5):
                load_xrow(h, 0)
            for i in range(4):      # blocks 0-3 = positions j 0..13 (half 0)
                load_wblk(i)
            for h in range(5):
                load_xrow(h, 1)
            for i in range(4, 7):
                load_wblk(i)
            load_xrow(5, 0)
            load_xrow(5, 1)
            for i in range(7, 9):
                load_wblk(i)
            load_xrow(6, 0)
            load_xrow(6, 1)
            load_wblk(9)
            load_xrow(7, 0)
            load_xrow(7, 1)
            for i in range(10, len(BLOCKS)):
                load_wblk(i)

            for i, n in enumerate(BLOCKS):
                t0 = t0s[i]
                wtile = wtiles[i]
                otile = opool.tile([B, n * O], f16, tag="ot")
                for tl in range(n):
                    t = t0 + tl
                    di, w2 = _pos_slot(t)
                    hf = 1 if w2 >= 18 else 0
                    ps = ppool.tile([B, O], f32, tag="ps")
                    for kc in range(NKC):
                        du, dv, reg = CHUNK_OFF[kc]
                        s = (w2 + dv - 18 * hf) * B + reg * HALF
                        xr = XH[di + du][hf]
                        rhs = wtile[:, (tl * NKC + kc) * O:(tl * NKC + kc + 1) * O]
                        if CHUNK_TAPS[kc][1] is not None:
                            lhsT = xr[:, s:s + B]
                        else:
                            lhsT = xr[0:C, s:s + B]
                            rhs = rhs[0:C]
                        nc.tensor.matmul(ps[:], lhsT, rhs,
                                         start=(kc == 0), stop=(kc == NKC - 1))
                    nc.vector.tensor_copy(otile[:, tl * O:(tl + 1) * O], ps[:])
                nc.gpsimd.dma_start(
                    out_ap[t0:t0 + n].rearrange("t b o -> b t o"),
                    otile[:].rearrange("b (t o) -> b t o", t=n),
                )

    nc.compile()
    _PROG_CACHE["nc"] = nc
    return nc


def _shifted(a, s):
    out = np.zeros_like(a)
    out[:, :a.shape[1] - s] = a[:, s:]
    return out


def _make_in_maps(x, weight):
    x_chwb = np.ascontiguousarray(np.asarray(x, np.float32).transpose(1, 2, 3, 0))
    x16 = x_chwb.astype(np.float16)
    w32 = np.asarray(weight, np.float32)
    in_maps = []
    for k in range(NCORES):
        xs = _build_xs(x16, k)
        xs1 = np.concatenate([xs, _shifted(xs, SH_W1)], axis=0)  # [128, XFLAT]
        xs2 = np.concatenate([xs, _shifted(xs, SH_W2)], axis=0)
        in_maps.append({
            "xs1": np.ascontiguousarray(xs1.reshape(128, XROWS, XW * B).transpose(1, 0, 2)),
            "xs2": np.ascontiguousarray(xs2.reshape(128, XROWS, XW * B).transpose(1, 0, 2)),
            "wt": _build_wt(w32, k),
        })
    return in_maps


def kernel(x, weight):
    from concourse.bass_utils import run_bass_kernel_spmd

    nc = _build_program()
    in_maps = _make_in_maps(x, weight)
    res = run_bass_kernel_spmd(nc, in_maps, core_ids=list(range(NCORES)))
    outs = [res.results[k]["out"].reshape(PPC, B, O) for k in range(NCORES)]
    return _assemble(outs)


# revision 21
# speedup vs baseline: 1.1005x; 1.1005x over previous
"""Locally-connected convolution (unshared weights) on 8 Trainium2 NeuronCores.

out[b,o,i,j] = sum_{c,u,v} x[b,c,i+u,j+v] * weight[i,j,o,c,u,v]
  x: [64, 64, 32, 32] f32, weight: [28, 28, 128, 64, 5, 5] f32 -> out [64, 128, 28, 28]

Strategy: each of the 784 output positions is an independent GEMM
[B=64, K=1600] x [K=1600, O=128].  Shard the 784 positions across 8 cores
(98 each, raster-contiguous).  Weights are reordered host-side into
K-chunk-major fp16 layout; x is pre-transposed/sheared host-side so that a
single SPMD program (identical AP offsets on every core) can slice 2-tap
K=128 lhsT tiles straight out of SBUF with zero on-device data movement.

K ordering (1600 = 25 taps x 64 ch, padded to 13 chunks x 128):
  chunk kc in [0,10): taps (u, ve) and (u, ve+1), u=kc//2, ve=2*(kc%2)
  chunk 10: taps (0,4),(1,4);  chunk 11: taps (2,4),(3,4)
  chunk 12: tap (4,4) alone (K=64)
The second tap of a chunk differs from the first by +1 column (chunks 0-9)
or +1 row (chunks 10-11) in the input plane, so partitions 64-127 of the
SBUF x-workspace hold a copy of x shifted by +1 column (region W1) or +1
row (region W2); a [128, 64] lhsT slice then covers both taps at once.
"""

import numpy as np

B, C, H, W = 64, 64, 32, 32
ROWS = COLS = 28
O, KH, KW = 128, 5, 5
NCORES = 8
PPC = (ROWS * COLS) // NCORES  # 98 positions per core
NKC = 13                       # K chunks of 128 (last is half)
KIN = 128
XROWS, XW = 8, 36              # sheared x grid: 8 input rows x 36 cols
XFLAT = XROWS * XW * B         # 18432 elements per channel-partition
SH_W1 = B                      # +1 column shift, in elements
SH_W2 = XW * B                 # +1 row shift
NPOS_BLK = 7
NBLK = PPC // NPOS_BLK         # 14 weight blocks per core
BLOCKS = [3, 4] + [7] * 13     # ramp-friendly block sizes (sum = 98)
FPOS = NKC * O                 # weight elements per partition per position

# kc -> (du, dv, region)  AP slot offset = ((di+du)*XW + (w2+dv))*B + region*XFLAT
CHUNK_OFF = [(kc // 2, 2 * (kc % 2), 0) for kc in range(10)] + [
    (0, 4, 1),
    (2, 4, 1),
    (4, 4, 0),  # lone tap (4,4): K=64, lower partitions only
]
# kc -> ((u0,v0), (u1,v1) or None)
CHUNK_TAPS = [((kc // 2, 2 * (kc % 2)), (kc // 2, 2 * (kc % 2) + 1)) for kc in range(10)] + [
    ((0, 4), (1, 4)),
    ((2, 4), (3, 4)),
    ((4, 4), None),
]


def _core_geom(k):
    p0 = PPC * k
    return p0 // COLS, p0 % COLS  # r0 (first input/output row), s0 in {0, 14}


def _pos_slot(t):
    """Relative position t in [0,98) -> (di, w2) grid coords shared by all cores."""
    di, jj = t // COLS, t % COLS
    return di, jj + (4 if jj >= 14 else 0)


def _build_xs(x_chwb, k):
    """x_chwb: [C,H,W,B] -> sheared per-core workspace [C, XROWS, XW, B]."""
    r0, s0 = _core_geom(k)
    xs = np.zeros((C, XROWS, XW, B), dtype=x_chwb.dtype)
    for h in range(XROWS):
        if s0 == 0:
            xs[:, h, 0:18] = x_chwb[:, r0 + h, 0:18]
            xs[:, h, 18:36] = x_chwb[:, r0 + h, 14:32]
        else:
            xs[:, h, 0:18] = x_chwb[:, r0 + h, 14:32]
            if r0 + h + 1 < H:
                xs[:, h, 18:36] = x_chwb[:, r0 + h + 1, 0:18]
    return xs.reshape(C, XFLAT)


def _abs_pos(k, t):
    p = PPC * k + t
    return p // COLS, p % COLS


def _build_wt(weight, k, dtype=np.float16):
    """weight [ROWS,COLS,O,C,KH,KW] -> per-core [KIN, PPC, NKC*O] in chunk layout."""
    ii, jj = zip(*[_abs_pos(k, t) for t in range(PPC)])
    wc = weight[list(ii), list(jj)]  # [PPC, O, C, KH, KW]
    uu = np.zeros((KIN, NKC), np.int64)
    vv = np.zeros((KIN, NKC), np.int64)
    cc = np.broadcast_to((np.arange(KIN) % C)[:, None], (KIN, NKC))
    valid = np.ones((KIN, NKC), bool)
    for kc in range(NKC):
        t0, t1 = CHUNK_TAPS[kc]
        uu[:C, kc], vv[:C, kc] = t0
        if t1 is None:
            valid[C:, kc] = False
        else:
            uu[C:, kc], vv[C:, kc] = t1
    # [PPC, O, KIN, NKC]
    wt = wc[:, :, cc, uu, vv]
    wt[:, :, ~valid] = 0
    # -> [KIN, PPC*NKC*O] position-major contiguous
    return np.ascontiguousarray(wt.transpose(2, 0, 3, 1)).reshape(KIN, PPC * NKC * O).astype(dtype)


def _emulate_core(xs_flat, wt, out_dtype=np.float32):
    """Pure-numpy emulation of the device program (mirrors AP arithmetic)."""
    wt = wt.reshape(KIN, PPC, NKC * O)
    out = np.zeros((PPC, B, O), out_dtype)
    for t in range(PPC):
        di, w2 = _pos_slot(t)
        acc = np.zeros((B, O), np.float32)
        for kc in range(NKC):
            du, dv, reg = CHUNK_OFF[kc]
            s = ((di + du) * XW + (w2 + dv)) * B
            lo = xs_flat[:, s:s + B]
            rhs = wt[:, t, kc * O:(kc + 1) * O]
            if CHUNK_TAPS[kc][1] is not None:
                sh = SH_W1 if reg == 0 else SH_W2
                assert s + sh + B <= XFLAT, (t, kc, s)
                hi = xs_flat[:, s + sh:s + sh + B]
                lhsT = np.concatenate([lo, hi], axis=0)
            else:
                lhsT = lo
                rhs = rhs[:C]
            acc += lhsT.astype(np.float32).T @ rhs.astype(np.float32)
        out[t] = acc
    return out


def _assemble(outs):
    """list of 8 per-core [PPC, B, O] -> [B, O, ROWS, COLS] f32."""
    full = np.concatenate([np.asarray(o, np.float32) for o in outs], axis=0)
    return np.ascontiguousarray(full.transpose(1, 2, 0)).reshape(B, O, ROWS, COLS)


_PROG_CACHE = {}


def _build_program():
    if "nc" in _PROG_CACHE:
        return _PROG_CACHE["nc"]
    import concourse.bass as bass
    import concourse.tile as tile
    from concourse import bacc, mybir

    f16, f32 = mybir.dt.float16, mybir.dt.float32
    XWB = XW * B      # one sheared input row, elements per partition
    HALF = 18 * B     # half a sheared row (positions j<14 touch only half 0)
    nc = bacc.Bacc("TRN2", target_bir_lowering=False, debug=False, num_devices=NCORES)
    xs1_d = nc.dram_tensor("xs1", [XROWS, 128, XWB], f16, kind="ExternalInput")
    xs2_d = nc.dram_tensor("xs2", [XROWS, 128, XWB], f16, kind="ExternalInput")
    wt_d = nc.dram_tensor("wt", [KIN, PPC * FPOS], f16, kind="ExternalInput")
    out_d = nc.dram_tensor("out", [PPC, B, O], f16, kind="ExternalOutput")

    with tile.TileContext(nc) as tc:
        with tc.tile_pool(name="xpool", bufs=1) as xpool, \
             tc.tile_pool(name="wpool", bufs=4) as wpool, \
             tc.tile_pool(name="opool", bufs=3) as opool, \
             tc.tile_pool(name="psum", bufs=8, space="PSUM") as ppool:
            xs1, xs2 = xs1_d.ap(), xs2_d.ap()
            XH = [[xpool.tile([128, 2 * HALF], f16, name=f"xh{h}_{hf}", tag=f"xh{h}_{hf}")
                   for hf in range(2)] for h in range(XROWS)]

            def load_xrow(h, hf):
                nc.sync.dma_start(XH[h][hf][:, 0:HALF], xs1[h, :, hf * HALF:(hf + 1) * HALF])
                nc.scalar.dma_start(XH[h][hf][:, HALF:2 * HALF], xs2[h, :, hf * HALF:(hf + 1) * HALF])

            wt_ap = wt_d.ap()
            out_ap = out_d.ap()
            eng = [nc.sync, nc.scalar]
            t0s = [sum(BLOCKS[:i]) for i in range(len(BLOCKS))]
            wtiles = [wpool.tile([KIN, n * FPOS], f16, name=f"wt{i}", tag="wt")
                      for i, n in enumerate(BLOCKS)]

            def load_wblk(i):
                # one DMA per position: finer deps (a position's matmuls only
                # wait for its own 416KB slice) and sub-µs PE waits keep HAM warm
                t0, n = t0s[i], BLOCKS[i]
                for tl in range(n):
                    t = t0 + tl
                    eng[t % 2].dma_start(
                        wtiles[i][:, tl * FPOS:(tl + 1) * FPOS],
                        wt_ap[:, t * FPOS:(t + 1) * FPOS])

            # emission order ~= per-ring FIFO order: first MMs (block 0: j=0..2,
            # row 0) need x rows 0-4 half 0 plus a 3-position weight tile.
            for h in range(5):
                load_xrow(h, 0)
            load_wblk(0)
            load_wblk(1)
            load_wblk(2)
            for h in range(5):
                load_xrow(h, 1)
            load_wblk(3)
            load_xrow(5, 0)
            load_xrow(5, 1)
            load_wblk(4)
            load_xrow(6, 0)
            load_xrow(6, 1)
            load_wblk(5)
            load_xrow(7, 0)
            load_xrow(7, 1)
            for i in range(6, len(BLOCKS)):
                load_wblk(i)

            for i, n in enumerate(BLOCKS):
                t0 = t0s[i]
                wtile = wtiles[i]
                otile = opool.tile([B, n * O], f16, tag="ot")
                for tl in range(n):
                    t = t0 + tl
                    di, w2 = _pos_slot(t)
                    hf = 1 if w2 >= 18 else 0
                    ps = ppool.tile([B, O], f32, tag="ps")
                    for kc in range(NKC):
                        du, dv, reg = CHUNK_OFF[kc]
                        s = (w2 + dv - 18 * hf) * B + reg * HALF
                        xr = XH[di + du][hf]
                        rhs = wtile[:, (tl * NKC + kc) * O:(tl * NKC + kc + 1) * O]
                        if CHUNK_TAPS[kc][1] is not None:
                            lhsT = xr[:, s:s + B]
                        else:
                            lhsT = xr[0:C, s:s + B]
                            rhs = rhs[0:C]
                        nc.tensor.matmul(ps[:], lhsT, rhs,
                                         start=(kc == 0), stop=(kc == NKC - 1))
                    nc.vector.tensor_copy(otile[:, tl * O:(tl + 1) * O], ps[:])
                nc.gpsimd.dma_start(
                    out_ap[t0:t0 + n].rearrange("t b o -> b t o"),
                    otile[:].rearrange("b (t o) -> b t o", t=n),
                )

    nc.compile()
    _PROG_CACHE["nc"] = nc
    return nc


def _shifted(a, s):
    out = np.zeros_like(a)
    out[:, :a.shape[1] - s] = a[:, s:]
    return out


def _make_in_maps(x, weight):
    x_chwb = np.ascontiguousarray(np.asarray(x, np.float32).transpose(1, 2, 3, 0))
    x16 = x_chwb.astype(np.float16)
    w32 = np.asarray(weight, np.float32)
    in_maps = []
    for k in range(NCORES):
        xs = _build_xs(x16, k)
        xs1 = np.concatenate([xs, _shifted(xs, SH_W1)], axis=0)  # [128, XFLAT]
        xs2 = np.concatenate([xs, _shifted(xs, SH_W2)], axis=0)
        in_maps.append({
            "xs1": np.ascontiguousarray(xs1.reshape(128, XROWS, XW * B).transpose(1, 0, 2)),
            "xs2": np.ascontiguousarray(xs2.reshape(128, XROWS, XW * B).transpose(1, 0, 2)),
            "wt": _build_wt(w32, k),
        })
    return in_maps


def kernel(x, weight):
    from concourse.bass_utils import run_bass_kernel_spmd

    nc = _build_program()
    in_maps = _make_in_maps(x, weight)
    res = run_bass_kernel_spmd(nc, in_maps, core_ids=list(range(NCORES)))
    outs = [res.results[k]["out"].reshape(PPC, B, O) for k in range(NCORES)]
    return _assemble(outs)
